# revision 7
# baseline (speedup 1.0000x reference)
"""Trainium2 Bass kernel for nn_DAWN_48069273977343 (moe_routing).

8 NeuronCores: core c -> batch c//2, sequence half c%2 (512 own tokens,
x rotated own-first).  K/V streams duplicated across the batch pair (no
collectives).  Exact top-K via K/8 rounds of DVE max8+match_replace.
Q/K/V round-trip through DRAM scratch to fit SBUF.  Host only
shards/preps, gathers, and reduces the aux scalar.
"""

import sys

for _p in ("/opt/trn_rl_repo",):
    if _p not in sys.path:
        sys.path.insert(0, _p)

import numpy as np
import concourse.mybir as mybir
from concourse.bass import Bass
from concourse.tile import TileContext
from concourse.masks import make_identity

dt = mybir.dt
Alu = mybir.AluOpType
Act = mybir.ActivationFunctionType
AxX = mybir.AxisListType.X

B, S, D = 4, 1024, 1024
DB = 64
N_QK, N_V, N_KNOW = 8192, 8192, 16384
K_QK, K_V, K_KNOW = 64, 64, 128
N_HEADS = 16
DH = D // N_HEADS
HALF = S // 2
NEG = -1.0e30
G_OWN = HALF // 128
G_ALL = S // 128


def _split_waits(nc, max_waits=1):
    for fn in nc.m.functions:
        for blk in fn.blocks:
            out, changed = [], False
            for inst in blk.instructions:
                si = inst.sync_info
                ws = list(si.on_wait) if (si is not None and si.on_wait) else []
                if len(ws) > max_waits:
                    changed = True
                    k = 0
                    while len(ws) > max_waits:
                        chunk, ws = ws[:max_waits], ws[max_waits:]
                        out.append(mybir.InstNoOp(
                            name=f"{inst.name}-wc{k}", engine=inst.engine,
                            sync_info=mybir.SyncInfo(on_wait=chunk, on_update=[]),
                            bass_nofuse=True))
                        k += 1
                    si.on_wait = ws
                out.append(inst)
            if changed:
                blk.instructions = out


def build_kernel():
    nc = Bass()
    f32 = dt.float32

    def din(name, shape):
        return nc.dram_tensor(name, shape, f32, kind="ExternalInput")

    xb = din("xb", [S, D])
    projcat = din("projcat", [D, 195])
    projbcat = din("projbcat", [128, 195])
    projkcat = din("projkcat", [D, 65])
    projkb = din("projkb", [128, 65])
    qk_normT = din("qk_normT", [DB, N_QK])
    v_normT = din("v_normT", [DB, N_V])
    know_normT = din("know_normT", [DB, N_KNOW])
    qk_enc = din("qk_enc", [N_QK, DB])
    v_enc = din("v_enc", [N_V, DB])
    know_enc = din("know_enc", [N_KNOW, DB])
    qk_dec = din("qk_dec", [DB, D])
    v_dec = din("v_dec", [DB, D])
    know_dec = din("know_dec", [DB, D])
    expand_O = din("expand_O", [D, D])
    mask_q = din("mask_q", [HALF, S])
    mask_t = din("mask_t", [S, HALF])
    ln1_s = din("ln1_s", [128, D]); ln1_b = din("ln1_b", [128, D])
    ln2_s = din("ln2_s", [128, D]); ln2_b = din("ln2_b", [128, D])

    out_x = nc.dram_tensor("out_x", [HALF, D], f32, kind="ExternalOutput")
    usage_qk_o = nc.dram_tensor("usage_qk_o", [1, N_QK], f32, kind="ExternalOutput")
    usage_v_o = nc.dram_tensor("usage_v_o", [1, N_V], f32, kind="ExternalOutput")
    usage_k_o = nc.dram_tensor("usage_k_o", [1, N_KNOW], f32, kind="ExternalOutput")

    # DRAM scratch
    Qd = nc.dram_tensor("Qd", [HALF, D], f32)
    Kd = nc.dram_tensor("Kd", [S, D], f32)
    Vd = nc.dram_tensor("Vd", [S, D], f32)
    Qr = Qd.rearrange("(g p) d -> p g d", p=128)
    Kr = Kd.rearrange("(g p) d -> p g d", p=128)
    Vr = Vd.rearrange("(g p) d -> p g d", p=128)
    xbr = xb.rearrange("(g p) d -> p g d", p=128)

    with TileContext(nc) as tc:
        with (
            tc.tile_pool(name="pp", bufs=1) as pp,
            tc.tile_pool(name="dsp", bufs=2) as dsp,
            tc.tile_pool(name="psum", bufs=1, space="PSUM") as ps,
        ):
            ident = pp.tile([128, 128], f32, tag="ident")
            make_identity(nc, ident[:])

            def tr128(src_ap, dst_ap, w=128):
                pt = ps.tile([128, 128], f32, tag="tr")
                nc.tensor.transpose(pt[:w, :], src_ap, ident[:])
                nc.any.tensor_copy(dst_ap, pt[:w, :])

            l1s = pp.tile([128, D], f32, tag="l1s"); nc.sync.dma_start(l1s[:], ln1_s[:])
            l1b = pp.tile([128, D], f32, tag="l1b"); nc.sync.dma_start(l1b[:], ln1_b[:])
            l2s = pp.tile([128, D], f32, tag="l2s"); nc.sync.dma_start(l2s[:], ln2_s[:])
            l2b = pp.tile([128, D], f32, tag="l2b"); nc.sync.dma_start(l2b[:], ln2_b[:])
            consts = pp.tile([128, 4], f32, tag="consts")
            nc.vector.memset(consts[:, 0:1], 1.0)
            nc.vector.memset(consts[:, 1:2], 0.0)
            nc.vector.memset(consts[:, 2:3], 1e-8)
            nc.vector.memset(consts[:, 3:4], 1.0 / D)
            ONE = consts[:, 0:1]; ZERO = consts[:, 1:2]; EPS8 = consts[:, 2:3]; RD = consts[:, 3:4]
            x2 = pp.tile([128, G_OWN, D], f32, tag="x2")

            def layer_norm_g(xg, sc, bi, outg, pool):
                mean = pool.tile([128, 1], f32, tag="ln_m")
                nc.vector.tensor_reduce(out=mean[:], in_=xg, op=Alu.add, axis=AxX)
                nc.vector.tensor_tensor(out=mean[:], in0=mean[:], in1=RD, op=Alu.mult)
                cen = pool.tile([128, D], f32, tag="ln_c")
                nc.vector.tensor_scalar(cen[:], xg, mean[:], None, op0=Alu.subtract)
                sq = pool.tile([128, D], f32, tag="ln_q")
                nc.scalar.activation(sq[:], cen[:], Act.Square)
                var = pool.tile([128, 1], f32, tag="ln_v")
                nc.vector.tensor_reduce(out=var[:], in_=sq[:], op=Alu.add, axis=AxX)
                nc.vector.tensor_tensor(out=var[:], in0=var[:], in1=RD, op=Alu.mult)
                nc.vector.tensor_scalar(var[:], var[:], 1e-6, None, op0=Alu.add)
                rstd = pool.tile([128, 1], f32, tag="ln_r")
                nc.scalar.activation(rstd[:], var[:], Act.Sqrt)
                nc.vector.reciprocal(rstd[:], rstd[:])
                nc.vector.tensor_scalar(cen[:], cen[:], rstd[:], None, op0=Alu.mult)
                nc.vector.tensor_tensor(out=cen[:], in0=cen[:], in1=sc, op=Alu.mult)
                nc.vector.tensor_tensor(out=outg, in0=cen[:], in1=bi, op=Alu.add)

            def proj_g(ng, w3, bvec, ncols, outg, pool):
                xT = pool.tile([128, 8, 128], f32, tag="ph_xT")
                for kd in range(8):
                    tr128(ng[:, kd * 128:(kd + 1) * 128], xT[:, kd])
                pt = ps.tile([128, 512], f32, tag="mm512")
                for kd in range(8):
                    nc.tensor.matmul(pt[:, :ncols], xT[:, kd], w3[:, kd],
                                     start=(kd == 0), stop=(kd == 7))
                nc.any.tensor_copy(outg, pt[:, :ncols])
                nc.vector.tensor_tensor(out=outg, in0=outg,
                                        in1=bvec, op=Alu.add)

            def stream(hsrc, hcol, n_groups, embT_d, enc_d, N, K, tau_col,
                       usage_groups, usage_out, decW, kscale, writer, pool):
                NT = N // 512
                enc_r = enc_d.rearrange("(k p) n -> p k n", p=128)
                for g in range(n_groups):
                    hT = pool.tile([DB, 128], f32, tag="st_hT")
                    tr128(hsrc[:, g, hcol:hcol + DB], hT[:], w=DB)
                    s = pool.tile([128, N_KNOW], f32, tag="st_s")
                    for nt in range(NT):
                        emb_t = dsp.tile([DB, 512], f32, tag="st_emb")
                        nc.sync.dma_start(emb_t[:], embT_d[:, nt * 512:(nt + 1) * 512])
                        pt = ps.tile([128, 512], f32, tag="mm512")
                        nc.tensor.matmul(pt[:], hT[:], emb_t[:], start=True, stop=True)
                        nc.any.tensor_copy(s[:, nt * 512:(nt + 1) * 512], pt[:])
                    scr = pool.tile([128, N_KNOW], f32, tag="st_scr")
                    nc.vector.tensor_copy(scr[:, :N], s[:, :N])
                    Lv = pool.tile([128, K_KNOW], f32, tag="st_L")
                    for r in range(K // 8):
                        nc.vector.max(out=Lv[:, r * 8:(r + 1) * 8], in_=scr[:, :N])
                        nc.vector.match_replace(out=scr[:, :N], in_to_replace=Lv[:, r * 8:(r + 1) * 8],
                                                in_values=scr[:, :N], imm_value=NEG)
                    rmax = Lv[:, 0:1]
                    theta = Lv[:, K - 1:K]
                    tau = hsrc[:, g, tau_col:tau_col + 1]
                    ntau = pool.tile([128, 1], f32, tag="st_ntau")
                    nc.vector.tensor_scalar(ntau[:], tau, -1.0, None, op0=Alu.mult)
                    if usage_out is not None and g < usage_groups:
                        nrm = pool.tile([128, 1], f32, tag="st_nrm")
                        nc.vector.tensor_scalar(nrm[:], rmax, -1.0, None, op0=Alu.mult)
                        gsU = pool.tile([128, 1], f32, tag="st_gsU")
                        nc.scalar.activation(scr[:, :N], s[:, :N], Act.Exp, bias=nrm[:], accum_out=gsU[:])
                        rU = pool.tile([128, 1], f32, tag="st_rU")
                        nc.vector.reciprocal(rU[:], gsU[:])
                        for nt in range(NT):
                            pt = ps.tile([1, 512], f32, tag="ups")
                            nc.tensor.matmul(pt[:], rU[:], scr[:, nt * 512:(nt + 1) * 512], start=True, stop=True)
                            ub = pool.tile([1, 512], f32, tag="st_usb")
                            nc.any.tensor_copy(ub[:], pt[:])
                            nc.gpsimd.dma_start(usage_out[:, nt * 512:(nt + 1) * 512], ub[:],
                                                accum_op=Alu.add)
                    negc = pool.tile([128, 1], f32, tag="st_negc")
                    nc.vector.memset(negc[:], NEG)
                    nc.vector.tensor_scalar(scr[:, :N], s[:, :N], theta, None, op0=Alu.is_lt)
                    nc.vector.tensor_scalar(scr[:, :N], scr[:, :N], negc[:], None, op0=Alu.mult)
                    nc.vector.tensor_tensor(out=s[:, :N], in0=s[:, :N], in1=scr[:, :N], op=Alu.add)
                    nc.scalar.activation(scr[:, :N], s[:, :N], Act.Exp, bias=ntau[:])
                    nc.vector.tensor_scalar(scr[:, :N], scr[:, :N], ONE, ZERO, op0=Alu.subtract, op1=Alu.max)
                    gsum = pool.tile([128, 1], f32, tag="st_gsum")
                    nc.vector.tensor_reduce(out=gsum[:], in_=scr[:, :N], op=Alu.add, axis=AxX)
                    egmax = pool.tile([128, 1], f32, tag="st_egmax")
                    nc.scalar.activation(egmax[:], rmax, Act.Exp, bias=ntau[:])
                    nc.vector.tensor_scalar(egmax[:], egmax[:], ONE, ZERO, op0=Alu.subtract, op1=Alu.max)
                    gstr = pool.tile([128, 1], f32, tag="st_gstr")
                    nc.scalar.activation(gstr[:], egmax[:], Act.Tanh)
                    nc.vector.tensor_tensor(out=gsum[:], in0=gsum[:], in1=EPS8, op=Alu.add)
                    scl = pool.tile([128, 1], f32, tag="st_scl")
                    nc.vector.reciprocal(scl[:], gsum[:])
                    nc.vector.tensor_tensor(out=scl[:], in0=scl[:], in1=gstr[:], op=Alu.mult)
                    if kscale is not None:
                        nc.vector.tensor_scalar(scl[:], scl[:], kscale, None, op0=Alu.mult)
                    hbp = ps.tile([DB, 128], f32, tag="acc")
                    for kt in range(N // 128):
                        et = ps.tile([128, 128], f32, tag="tr")
                        nc.tensor.transpose(et[:], scr[:, kt * 128:(kt + 1) * 128], ident[:])
                        es = pool.tile([128, 128], f32, tag="st_egs")
                        nc.any.tensor_copy(es[:], et[:])
                        enc_t = dsp.tile([128, DB], f32, tag="st_enc")
                        nc.sync.dma_start(enc_t[:], enc_r[:, kt])
                        nc.tensor.matmul(hbp[:], enc_t[:], es[:], start=(kt == 0), stop=(kt == N // 128 - 1))
                    hbT = pool.tile([DB, 128], f32, tag="st_hbT")
                    nc.any.tensor_copy(hbT[:], hbp[:])
                    for ot in range(D // 512):
                        pt = ps.tile([128, 512], f32, tag="mm512")
                        nc.tensor.matmul(pt[:], hbT[:], decW[:, ot * 512:(ot + 1) * 512], start=True, stop=True)
                        writer(g, ot, pt, scl, pool)

            # ============ circuit 1 ============
            with tc.tile_pool(name="pQV", bufs=1) as pQV:
                hall = pQV.tile([128, G_ALL, 195], f32, tag="hall")
                with tc.tile_pool(name="pA", bufs=1) as pA:
                    projc = pA.tile([128, 8, 195], f32, tag="projc")
                    nc.sync.dma_start(projc[:], projcat.rearrange("(k p) n -> p k n", p=128))
                    projcb = pA.tile([128, 195], f32, tag="projcb")
                    nc.sync.dma_start(projcb[:], projbcat[:])
                    for g in range(G_ALL):
                        xg = pA.tile([128, D], f32, tag="xg")
                        nc.sync.dma_start(xg[:], xbr[:, g])
                        ng = pA.tile([128, D], f32, tag="ng")
                        layer_norm_g(xg[:], l1s[:], l1b[:], ng[:], pA)
                        proj_g(ng[:], projc, projcb[:, :195], 195, hall[:, g], pA)

                with tc.tile_pool(name="pS", bufs=1) as pS:
                    qkD = pS.tile([DB, D], f32, tag="qkD"); nc.sync.dma_start(qkD[:], qk_dec[:])
                    vD = pS.tile([DB, D], f32, tag="vD"); nc.sync.dma_start(vD[:], v_dec[:])

                    def wr(dram_r):
                        def w(g, ot, pt, scl, pool):
                            ob = pool.tile([128, 512], f32, tag="st_ob")
                            nc.vector.tensor_scalar(ob[:], pt[:], scl[:], None, op0=Alu.mult)
                            nc.sync.dma_start(dram_r[:, g, ot * 512:(ot + 1) * 512], ob[:])
                        return w

                    stream(hall, 0, G_OWN, qk_normT, qk_enc, N_QK, K_QK, 192,
                           G_OWN, usage_qk_o, qkD, 1.0 / 8.0, wr(Qr), pS)
                    stream(hall, DB, G_ALL, qk_normT, qk_enc, N_QK, K_QK, 193,
                           0, None, qkD, None, wr(Kr), pS)
                    stream(hall, 2 * DB, G_ALL, v_normT, v_enc, N_V, K_V, 194,
                           G_OWN, usage_v_o, vD, None, wr(Vr), pS)

            # ============ attention ============
            with tc.tile_pool(name="pT", bufs=1) as pT:
                mqr = mask_q.rearrange("(g p) k -> p g k", p=128)
                mtr = mask_t.rearrange("(g p) q -> p g q", p=128)
                outT_all = pT.tile([128, 8, HALF], f32, tag="outT_all")
                for h in range(N_HEADS):
                    c0 = h * DH
                    QTa = pT.tile([DB + 1, G_OWN, 128], f32, tag="QTa")
                    KTa = pT.tile([DB + 1, G_ALL, 128], f32, tag="KTa")
                    for g in range(G_OWN):
                        qs = dsp.tile([128, DH], f32, tag="qs")
                        nc.sync.dma_start(qs[:], Qr[:, g, c0:c0 + DH])
                        tr128(qs[:], QTa[:DB, g], w=DB)
                    for g in range(G_ALL):
                        ks = dsp.tile([128, DH], f32, tag="ks")
                        nc.sync.dma_start(ks[:], Kr[:, g, c0:c0 + DH])
                        tr128(ks[:], KTa[:DB, g], w=DB)
                        nc.vector.memset(KTa[DB:DB + 1, g], 1.0)
                    for g in range(G_OWN):
                        att = pT.tile([128, S], f32, tag="at1")
                        for nt in range(S // 512):
                            pt = ps.tile([128, 512], f32, tag="mm512")
                            for kg in range(4):
                                nc.tensor.matmul(pt[:, kg * 128:(kg + 1) * 128], QTa[:DB, g],
                                                 KTa[:DB, nt * 4 + kg], start=True, stop=True)
                            nc.any.tensor_copy(att[:, nt * 512:(nt + 1) * 512], pt[:])
                        mqt = pT.tile([128, S], f32, tag="mqt")
                        nc.sync.dma_start(mqt[:], mqr[:, g])
                        nc.vector.tensor_tensor(out=att[:], in0=att[:], in1=mqt[:], op=Alu.add)
                        mrow = pT.tile([128, 1], f32, tag="at_m")
                        nc.vector.tensor_reduce(out=mrow[:], in_=att[:], op=Alu.max, axis=AxX)
                        nm = pT.tile([128, 1], f32, tag="at_nm")
                        nc.vector.tensor_scalar(nm[:], mrow[:], -1.0, None, op0=Alu.mult)
                        lrow = pT.tile([128, 1], f32, tag="at_l")
                        ex = pT.tile([128, S], f32, tag="at_ex")
                        nc.scalar.activation(ex[:], att[:], Act.Exp, bias=nm[:], accum_out=lrow[:])
                        nc.scalar.activation(lrow[:], lrow[:], Act.Ln)
                        nc.vector.tensor_tensor(out=mrow[:], in0=mrow[:], in1=lrow[:], op=Alu.add)
                        nc.vector.tensor_scalar(mrow[:], mrow[:], -1.0, None, op0=Alu.mult)
                        mp = ps.tile([128, 128], f32, tag="tr")
                        nc.tensor.transpose(mp[:], mrow[:].to_broadcast([128, 128]), ident[:])
                        nc.any.tensor_copy(QTa[DB:DB + 1, g], mp[:1])
                    op_ps = ps.tile([DB, HALF], f32, tag="acc")
                    for kg in range(G_ALL):
                        pt = ps.tile([128, HALF], f32, tag="mm512")
                        for g in range(G_OWN):
                            nc.tensor.matmul(pt[:, g * 128:(g + 1) * 128], KTa[:, kg], QTa[:, g],
                                             start=True, stop=True)
                        atT = pT.tile([128, HALF], f32, tag="at2")
                        mtt = pT.tile([128, HALF], f32, tag="mtt")
                        nc.sync.dma_start(mtt[:], mtr[:, kg])
                        nc.vector.tensor_tensor(out=atT[:], in0=pt[:], in1=mtt[:], op=Alu.add)
                        nc.scalar.activation(atT[:], atT[:], Act.Exp)
                        vs = dsp.tile([128, DH], f32, tag="vs")
                        nc.sync.dma_start(vs[:], Vr[:, kg, c0:c0 + DH])
                        nc.tensor.matmul(op_ps[:], vs[:], atT[:],
                                         start=(kg == 0), stop=(kg == G_ALL - 1))
                    nc.any.tensor_copy(outT_all[c0 % 128:c0 % 128 + DH, c0 // 128], op_ps[:])
                expO_r = expand_O.rearrange("(k p) n -> p k n", p=128)
                for g in range(G_OWN):
                    for ot in range(D // 512):
                        pt = ps.tile([128, 512], f32, tag="mm512")
                        for kd in range(8):
                            eo = dsp.tile([128, 512], f32, tag="eo")
                            nc.sync.dma_start(eo[:], expO_r[:, kd, ot * 512:(ot + 1) * 512])
                            nc.tensor.matmul(pt[:], outT_all[:, kd, g * 128:(g + 1) * 128], eo[:],
                                             start=(kd == 0), stop=(kd == 7))
                        xo = pT.tile([128, 512], f32, tag="xo")
                        nc.sync.dma_start(xo[:], xbr[:, g, ot * 512:(ot + 1) * 512])
                        ob = pT.tile([128, 512], f32, tag="ob")
                        nc.any.tensor_copy(ob[:], pt[:])
                        nc.vector.tensor_tensor(out=x2[:, g, ot * 512:(ot + 1) * 512],
                                                in0=ob[:], in1=xo[:], op=Alu.add)

            # ============ circuit 2 ============
            with tc.tile_pool(name="pK", bufs=1) as pK:
                projk = pK.tile([128, 8, 65], f32, tag="projk")
                nc.sync.dma_start(projk[:], projkcat.rearrange("(k p) n -> p k n", p=128))
                projkbb = pK.tile([128, 65], f32, tag="projkbb")
                nc.sync.dma_start(projkbb[:], projkb[:])
                kwD = pK.tile([DB, D], f32, tag="kwD")
                nc.sync.dma_start(kwD[:], know_dec[:])
                hk = pK.tile([128, G_OWN, 65], f32, tag="hk")
                for g in range(G_OWN):
                    ng = pK.tile([128, D], f32, tag="ng2")
                    layer_norm_g(x2[:, g], l2s[:], l2b[:], ng[:], pK)
                    proj_g(ng[:], projk, projkbb[:, :65], 65, hk[:, g], pK)

                def wk(g, ot, pt, scl, pool):
                    ob = pool.tile([128, 512], f32, tag="k_ob")
                    nc.vector.tensor_scalar(ob[:], pt[:], scl[:], None, op0=Alu.mult)
                    nc.vector.tensor_tensor(out=ob[:], in0=ob[:],
                                            in1=x2[:, g, ot * 512:(ot + 1) * 512], op=Alu.add)
                    nc.sync.dma_start(out_x.rearrange("(g p) d -> p g d", p=128)[:, g, ot * 512:(ot + 1) * 512],
                                      ob[:])

                stream(hk, 0, G_OWN, know_normT, know_enc, N_KNOW, K_KNOW, 64,
                       G_OWN, usage_k_o, kwD, None, wk, pK)

    _split_waits(nc)
    return nc


# ======================================================================
# Host wrapper
# ======================================================================
_NC_CACHE = {}


def kernel(**inputs):
    from concourse.bass_utils import run_bass_kernel_spmd

    x = np.asarray(inputs["x"], np.float32)

    def unit(a):
        a = np.asarray(a, np.float32)
        return a / (np.linalg.norm(a, axis=-1, keepdims=True) + 1e-8)

    common = dict(
        projcat=np.concatenate([inputs["proj_attn_k"], inputs["tau_attn_k"]], axis=1).astype(np.float32),
        projbcat=np.ascontiguousarray(np.broadcast_to(np.concatenate([inputs["proj_attn_b"], inputs["tau_attn_b"]])[None], (128, 195))).astype(np.float32),
        projkcat=np.concatenate([inputs["proj_know_k"], inputs["tau_know_k"]], axis=1).astype(np.float32),
        projkb=np.ascontiguousarray(np.broadcast_to(np.concatenate([inputs["proj_know_b"], inputs["tau_know_b"]])[None], (128, 65))).astype(np.float32),
        qk_normT=unit(inputs["qk_emb"]).T.copy(),
        v_normT=unit(inputs["v_emb"]).T.copy(),
        know_normT=unit(inputs["know_emb"]).T.copy(),
        qk_enc=np.asarray(inputs["qk_w_enc"], np.float32),
        v_enc=np.asarray(inputs["v_w_enc"], np.float32),
        know_enc=np.asarray(inputs["know_w_enc"], np.float32),
        qk_dec=np.asarray(inputs["qk_w_dec"], np.float32),
        v_dec=np.asarray(inputs["v_w_dec"], np.float32),
        know_dec=np.asarray(inputs["know_w_dec"], np.float32),
        expand_O=np.asarray(inputs["expand_O"], np.float32),
        ln1_s=np.ascontiguousarray(np.broadcast_to(np.asarray(inputs["norm1_scale"], np.float32)[None], (128, D))),
        ln1_b=np.ascontiguousarray(np.broadcast_to(np.asarray(inputs["norm1_bias"], np.float32)[None], (128, D))),
        ln2_s=np.ascontiguousarray(np.broadcast_to(np.asarray(inputs["norm2_scale"], np.float32)[None], (128, D))),
        ln2_b=np.ascontiguousarray(np.broadcast_to(np.asarray(inputs["norm2_bias"], np.float32)[None], (128, D))),
    )

    in_maps = []
    for c in range(8):
        b, hf = c // 2, c % 2
        xb_rot = np.roll(x[b], -hf * HALF, axis=0).copy()
        gk = (hf * HALF + np.arange(S)) % S
        gq = hf * HALF + np.arange(HALF)
        mq = np.where(gk[None, :] <= gq[:, None], 0.0, NEG).astype(np.float32)
        m = dict(common)
        m.update(xb=xb_rot, mask_q=mq, mask_t=mq.T.copy())
        in_maps.append(m)

    if "k" not in _NC_CACHE:
        _NC_CACHE["k"] = build_kernel()
    nc = _NC_CACHE["k"]

    res = run_bass_kernel_spmd(nc, in_maps, core_ids=list(range(8)))
    outs = res.results

    xo = np.empty((B, S, D), np.float32)
    u_qk = np.zeros(N_QK, np.float64)
    u_v = np.zeros(N_V, np.float64)
    u_k = np.zeros(N_KNOW, np.float64)
    for c in range(8):
        b, hf = c // 2, c % 2
        xo[b, hf * HALF:(hf + 1) * HALF] = outs[c]["out_x"]
        u_qk += outs[c]["usage_qk_o"][0]
        u_v += outs[c]["usage_v_o"][0]
        u_k += outs[c]["usage_k_o"][0]
    u_qk = (u_qk / (B * S)).astype(np.float32)
    u_v = (u_v / (B * S)).astype(np.float32)
    u_k = (u_k / (B * S)).astype(np.float32)
    a_aux = (((u_qk - 1.0 / N_QK) ** 2).sum() * N_QK * 3
             + ((u_v - 1.0 / N_V) ** 2).sum() * N_V)
    k_aux = ((u_k - 1.0 / N_KNOW) ** 2).sum() * N_KNOW
    return xo, np.float32(a_aux + k_aux)
